# revision 19
# baseline (speedup 1.0000x reference)
"""Trainium2 Bass kernel for a GPT-2 style transformer block (nn_Block_16690242913196).

v3 sharding strategy (8 NeuronCores, identical SPMD program):
  - Re-shard: core c owns 256 tokens of batch 0 ([256c, 256c+256)) and the
    matching 256 tokens of batch 1.  Token-parallel phases (LN, QKV, proj,
    MLP) use this layout; attention is head-parallel (2 heads/core).
  - QKV is computed token-parallel (full attn_w on every core, fp8
    DoubleRow), then one small fp8 AllToAll per batch redistributes
    q/k/v to the head-parallel layout.  A second fp8 AllToAll per batch
    returns attention outputs to token-parallel for the projection.
  - The two batches are software-pipelined: LN1(b1)+QKV(b1) overlap
    A2A#1(b0), attention(b0) overlaps A2A#1(b1), attention(b1) overlaps
    A2A#2(b0).  All collective transit tiles are single big DMAs.
  - LN gains/biases are folded into the following matmul weights on the
    host; on-chip LN is a pure normalize.  aw/pw are pre-scaled x256 on
    the host so fp8(e4m3) sees normal-range values; 1/256 is folded into
    the PSUM-evacuation activations.
  - fp8 + DoubleRow for QKV, attention*V and proj.  The attention-V
    stationary carries 64 replicated ones-columns so the softmax
    denominator lands broadcast across 64 PSUM partitions; 1/denom is
    computed as square(rsqrt) on the ACT engine (no single-lane DVE
    reciprocals, no broadcast matmul).  Scores q@k and the MLP stay at
    bf16-class precision; residual spine and accumulation are fp32.
"""

import numpy as np
import ml_dtypes

P = 128
B, S, D, H = 2, 2048, 1024, 16
DH = D // H          # 64
DI = 4 * D           # 4096
EPS = 1e-5
NCORES = 8
TOKH = 256           # tokens per batch per core
TOK = 2 * TOKH       # 512 tokens per core
KD = D // P          # 8
KDI = DI // P        # 32
HL = H // NCORES     # 2 local heads
RG = [list(range(NCORES))]

_CACHED_NC = None


def build_nc():
    import concourse.bacc as bacc
    import concourse.tile as tile
    import concourse.mybir as mybir
    from contextlib import ExitStack

    dt = mybir.dt
    f32, bf16, f32r, f8 = dt.float32, dt.bfloat16, dt.float32r, dt.float8e4
    AF = mybir.ActivationFunctionType
    OP = mybir.AluOpType
    DR = mybir.MatmulPerfMode.DoubleRow

    nc = bacc.Bacc("TRN2", target_bir_lowering=False, debug=False,
                   num_devices=NCORES)

    # ---- kernel I/O (per-core shapes) ----
    xT = nc.dram_tensor("xT", [D, TOK], f32r, kind="ExternalInput").ap()
    aw = nc.dram_tensor("aw", [P, KD, 3 * D], f8, kind="ExternalInput").ap()
    qkb = nc.dram_tensor("qkb", [P, 2, NCORES], f32, kind="ExternalInput").ap()
    pw = nc.dram_tensor("pw", [P, KD, D], f8, kind="ExternalInput").ap()
    pb = nc.dram_tensor("pb", [P, KD], f32, kind="ExternalInput").ap()
    fw = nc.dram_tensor("fw", [KDI, P, KD, P], bf16, kind="ExternalInput").ap()
    fb = nc.dram_tensor("fb", [P, KDI], f32, kind="ExternalInput").ap()
    gw = nc.dram_tensor("gw", [KD, P, KDI, P], bf16, kind="ExternalInput").ap()
    gb = nc.dram_tensor("gb", [P, KD], f32, kind="ExternalInput").ap()
    mk = nc.dram_tensor("mk", [P, 2, TOKH], f8, kind="ExternalInput").ap()
    mkd = nc.dram_tensor("mkd", [P, 2, TOK], f8, kind="ExternalInput").ap()
    idm = nc.dram_tensor("idm", [P, P], f8, kind="ExternalInput").ap()
    outT = nc.dram_tensor("outT", [D, TOK], f32, kind="ExternalOutput").ap()

    with tile.TileContext(nc) as tc, ExitStack() as ctx:
        const = ctx.enter_context(tc.tile_pool(name="const", bufs=1))
        dram = ctx.enter_context(tc.tile_pool(name="dram", bufs=1, space="DRAM"))
        psum = ctx.enter_context(tc.tile_pool(name="psum", bufs=1, space="PSUM"))
        rows = ctx.enter_context(tc.tile_pool(name="rows", bufs=6))
        sqp = ctx.enter_context(tc.tile_pool(name="sqp", bufs=3))
        lnt = ctx.enter_context(tc.tile_pool(name="lnt", bufs=3))
        res = ctx.enter_context(tc.tile_pool(name="res", bufs=1))
        s1 = ExitStack()
        ares = s1.enter_context(tc.tile_pool(name="ares", bufs=1))
        awp = s1.enter_context(tc.tile_pool(name="awp", bufs=1))
        wp = s1.enter_context(tc.tile_pool(name="wp", bufs=4))

        # ---- constants in SBUF (xT first: LN1 needs it before aw/pw) ----
        xT_sb = res.tile([P, KD, TOK], f32r)
        for k in range(KD):
            nc.sync.dma_start(
                xT_sb[:, k, :],
                xT[P * k:P * (k + 1), :])
        aw_sb = awp.tile([P, KD, 3 * D], f8)
        nc.sync.dma_start(aw_sb, aw)
        pw_sb = awp.tile([P, KD, D], f8)
        nc.sync.dma_start(pw_sb, pw)
        qkb_sb = const.tile([P, 2, NCORES], f32)
        nc.sync.dma_start(qkb_sb, qkb)
        pb_sb = const.tile([P, KD], f32)
        nc.sync.dma_start(pb_sb, pb)
        fb_sb = const.tile([P, KDI], f32)
        nc.sync.dma_start(fb_sb, fb)
        gb_sb = const.tile([P, KD], f32)
        nc.sync.dma_start(gb_sb, gb)
        mkc = const.tile([P, 2, TOKH], f8)
        nc.sync.dma_start(mkc, mk)
        mkd_sb = const.tile([P, 2, TOK], f8)
        nc.sync.dma_start(mkd_sb, mkd)
        idm_sb = const.tile([P, P], f8)
        nc.sync.dma_start(idm_sb, idm)
        ones_cf = const.tile([P, 1], f32)
        nc.vector.memset(ones_cf, 1.0)
        ones_c = const.tile([P, 1], f32r)
        nc.vector.tensor_copy(ones_c, ones_cf)
        ones_rf = const.tile([1, P], f32)
        nc.vector.memset(ones_rf, 1.0)
        ones_r = const.tile([1, P], f32r)
        nc.vector.tensor_copy(ones_r, ones_rf)
        eps_sb = const.tile([1, 1], f32)
        nc.vector.memset(eps_sb, EPS)

        # collective bounce buffers (per batch)
        CSZ = P * TOKH  # bytes per q/k/v slot (fp8)
        cc1_in = [dram.tile([NCORES, 3, CSZ], f8, name=f"cc1i{b}")
                  for b in range(B)]
        cc1_out = [dram.tile([NCORES, 3, CSZ], f8, name=f"cc1o{b}")
                   for b in range(B)]
        cc2_in = [dram.tile([NCORES, P, TOKH], f8, name=f"cc2i{b}")
                  for b in range(B)]
        cc2_out = [dram.tile([NCORES, P, TOKH], f8, name=f"cc2o{b}")
                   for b in range(B)]

        def emit_ln(src_fn, dst_fn, tag, ts, W):
            """Pure layernorm (g/b folded into downstream weights) over
            token-column slice ts of width W: stats over the partition
            (feature) axis via f32r ones-matmuls; x^2 on the ACT engine."""
            sx = psum.tile([1, W], f32, tag="qk", bufs=2)
            sxx = psum.tile([1, W], f32, tag="acc", bufs=2)
            for k in range(KD):
                sq = sqp.tile([P, W], f32r, tag="sq")
                nc.vector.tensor_tensor(out=sq, in0=src_fn(k)[:, ts],
                                        in1=src_fn(k)[:, ts], op=OP.mult)
                nc.tensor.matmul(sx, ones_c, src_fn(k)[:, ts],
                                 start=(k == 0), stop=(k == KD - 1))
                nc.tensor.matmul(sxx, ones_c, sq,
                                 start=(k == 0), stop=(k == KD - 1))
            mu = rows.tile([1, W], f32r, tag="row")
            nc.vector.tensor_scalar_mul(mu, sx, 1.0 / D)
            m2 = rows.tile([1, W], f32, tag="row")
            nc.vector.tensor_scalar_mul(m2, sxx, 1.0 / D)
            var = rows.tile([1, W], f32, tag="row")
            nc.vector.tensor_tensor(out=var, in0=mu, in1=mu, op=OP.mult)
            nc.vector.tensor_tensor(out=var, in0=m2, in1=var, op=OP.subtract)
            rstd = rows.tile([1, W], f32r, tag="row")
            with nc.allow_low_precision(reason="rsqrt table rstd"):
                nc.scalar.activation(rstd, var, AF.Abs_reciprocal_sqrt,
                                     bias=eps_sb[:])
            mub = psum.tile([P, W], f32, tag="sc", bufs=2)
            nc.tensor.matmul(mub, ones_r, mu, start=True, stop=True)
            rsb = psum.tile([P, W], f32, tag="sc", bufs=2)
            nc.tensor.matmul(rsb, ones_r, rstd, start=True, stop=True)
            for k in range(KD):
                t1 = lnt.tile([P, W], f32, tag=tag)
                nc.vector.tensor_tensor(out=t1, in0=src_fn(k)[:, ts], in1=mub,
                                        op=OP.subtract)
                with nc.allow_low_precision(reason="ln out quant"):
                    nc.vector.tensor_tensor(out=dst_fn(k)[:, ts], in1=rsb,
                                            in0=t1, op=OP.mult)

        xn8 = ares.tile([P, KD, TOK], f8)
        qS = [None] * B
        kS = [None] * B
        vS = [None] * B
        qF = [None] * B
        kF = [None] * B
        vF = [None] * B
        aT = [None] * B
        aF = [None] * B

        # ---- per batch: LN1 -> QKV -> A2A#1 ----
        for b in range(B):
            tb = slice(TOKH * b, TOKH * (b + 1))
            with nc.named_scope(f"ln1_{b}"):
                emit_ln(lambda k: xT_sb[:, k, :], lambda k: xn8[:, k, :],
                        "ln1", tb, TOKH)
            with nc.named_scope(f"qkv{b}"):
                qS[b] = ares.tile([P, NCORES, TOKH], f8, name=f"qS{b}")
                kS[b] = ares.tile([P, NCORES, TOKH], f8, name=f"kS{b}")
                vS[b] = ares.tile([P, 2, 2, 4 * P], f8, name=f"vS{b}")
                for r in range(NCORES):
                    ps = psum.tile([P, TOK], f32, tag="qk", bufs=2)
                    for which in range(2):
                        cb = which * D + P * r
                        for kk in range(KD // 2):
                            k2 = slice(2 * kk, 2 * kk + 2)
                            nc.tensor.matmul(
                                ps[:, TOKH * which:TOKH * (which + 1)],
                                aw_sb[:, k2, cb:cb + P],
                                xn8[:, k2, tb],
                                start=(kk == 0), stop=(kk == KD // 2 - 1),
                                perf_mode=DR, skip_group_check=True)
                    with nc.allow_low_precision(reason="q fp8"):
                        nc.scalar.activation(qS[b][:, r, :], ps[:, 0:TOKH],
                                             AF.Identity,
                                             bias=qkb_sb[:, 0, r:r + 1],
                                             scale=1.0 / 256.0)
                    with nc.allow_low_precision(reason="k fp8"):
                        nc.scalar.activation(kS[b][:, r, :], ps[:, TOKH:TOK],
                                             AF.Identity,
                                             bias=qkb_sb[:, 1, r:r + 1],
                                             scale=1.0 / 256.0)
                for g in range(2):       # dest groups of 4
                    for t in range(2):   # 128-token subchunks
                        psv = psum.tile([P, TOK], f32, tag="qk", bufs=2)
                        tsl = slice(TOKH * b + P * t, TOKH * b + P * (t + 1))
                        for kk in range(KD // 2):
                            k2 = slice(2 * kk, 2 * kk + 2)
                            nc.tensor.matmul(
                                psv, xn8[:, k2, tsl],
                                aw_sb[:, k2, 2 * D + 512 * g:2 * D + 512 * (g + 1)],
                                start=(kk == 0), stop=(kk == KD // 2 - 1),
                                perf_mode=DR)
                        with nc.allow_low_precision(reason="v fp8"):
                            nc.scalar.activation(vS[b][:, g, t, :], psv,
                                                 AF.Identity,
                                                 scale=1.0 / 256.0)
                # sends: one big DMA each for q, k; one per v dest-group
                nc.sync.dma_start(
                    cc1_in[b][:, 0, :].rearrange("r (p c) -> p r c", p=P),
                    qS[b])
                nc.sync.dma_start(
                    cc1_in[b][:, 1, :].rearrange("r (p c) -> p r c", p=P),
                    kS[b])
                ccv = cc1_in[b][:, 2, :].rearrange("d (t p f) -> p d t f",
                                                   t=2, p=P)
                for g in range(2):
                    for t in range(2):
                        nc.sync.dma_start(
                            ccv[:, 4 * g:4 * (g + 1), t, :],
                            vS[b][:, g, t, :].rearrange("p (d f) -> p d f",
                                                        d=4))
            with nc.named_scope(f"cc1_{b}"):
                nc.gpsimd.collective_compute(
                    "AllToAll", OP.bypass, replica_groups=RG,
                    ins=[cc1_in[b][:].opt()], outs=[cc1_out[b][:].opt()])

        def recv_qkv(b):
            qF[b] = ares.tile([P, NCORES, TOKH], f8, name=f"qF{b}")
            nc.gpsimd.dma_start(
                qF[b], cc1_out[b][:, 0, :].rearrange("r (p c) -> p r c", p=P))
            kF[b] = ares.tile([P, NCORES, TOKH], f8, name=f"kF{b}")
            nc.gpsimd.dma_start(
                kF[b], cc1_out[b][:, 1, :].rearrange("r (p c) -> p r c", p=P))
            vF[b] = ares.tile([P, NCORES, HL, 2, P], f8, name=f"vF{b}")
            vsrc = cc1_out[b][:, 2, :].rearrange("r (t p h d) -> p r t h d",
                                                 t=2, p=P, h=HL)
            for h in range(HL):
                nc.vector.memset(vF[b][:, :, h, :, DH:P], 1.0)
                for t in range(2):
                    nc.gpsimd.dma_start(vF[b][:, :, h, t, 0:DH],
                                        vsrc[:, :, t, h, :])

        def emit_attn(b, post_qc=None):
            aT[b] = ares.tile([P, NCORES, TOKH], f8, name=f"aT{b}")
            with nc.named_scope(f"attn{b}"):
                for qcp in range(NCORES // 2):
                    if post_qc is not None:
                        post_qc(qcp)
                    c0, c1 = 2 * qcp, 2 * qcp + 1
                    qp = slice(P * c0 // 2, 0)  # unused placeholder
                    for h in range(HL):
                        hb = DH * h
                        acc = psum.tile([P, TOK], f32, tag="acc", bufs=2,
                                        name=f"acc{b}_{qcp}_{h}")
                        # full-pair blocks: r2 < 2*qcp, both chunks attend
                        for r2 in range(c0):
                            sc4 = psum.tile([P, 2 * TOK], f32, tag="sc",
                                            bufs=2)
                            for j in range(2):
                                nc.tensor.matmul(
                                    sc4[:, TOK * j:TOK * (j + 1)],
                                    kF[b][hb:hb + DH, r2, P * j:P * (j + 1)],
                                    qF[b][hb:hb + DH, c0:c0 + 2, :],
                                    start=True, stop=True,
                                    skip_group_check=True)
                            w = wp.tile([P, 2 * TOK], f8, tag="w")
                            with nc.allow_low_precision(reason="probs fp8"):
                                nc.scalar.activation(w, sc4, AF.Exp,
                                                     scale=0.125)
                            nc.tensor.matmul(
                                acc, vF[b][:, r2, h],
                                w.rearrange("p (t c) -> p t c", t=2),
                                start=(r2 == 0), stop=False,
                                perf_mode=DR, skip_group_check=True)
                        # diagA: r2 = 2*qcp -- diagonal for chunk c0,
                        # fully attended by chunk c1
                        r2 = c0
                        sc4 = psum.tile([P, 2 * TOK], f32, tag="sc", bufs=2)
                        for j in range(2):
                            nc.tensor.matmul(
                                sc4[:, TOK * j:TOK * (j + 1)],
                                idm_sb, mkd_sb[:, j, :],
                                start=True, stop=False,
                                skip_group_check=True)
                            nc.tensor.matmul(
                                sc4[:, TOK * j:TOK * (j + 1)],
                                kF[b][hb:hb + DH, r2, P * j:P * (j + 1)],
                                qF[b][hb:hb + DH, c0:c0 + 2, :],
                                start=False, stop=True,
                                skip_group_check=True)
                        w = wp.tile([P, 2 * TOK], f8, tag="w")
                        with nc.allow_low_precision(reason="probs fp8"):
                            nc.scalar.activation(w, sc4, AF.Exp, scale=0.125)
                        nc.tensor.matmul(
                            acc, vF[b][:, r2, h],
                            w.rearrange("p (t c) -> p t c", t=2),
                            start=(r2 == 0), stop=False,
                            perf_mode=DR, skip_group_check=True)
                        # diagB: r2 = 2*qcp+1 -- diagonal for chunk c1 only
                        r2 = c1
                        scd = psum.tile([P, 2 * TOK], f32, tag="sc", bufs=2)
                        nc.tensor.matmul(
                            scd[:, 0:TOK],
                            idm_sb, mkc.rearrange("p m q -> p (m q)"),
                            start=True, stop=False,
                            skip_group_check=True)
                        for j in range(2):
                            nc.tensor.matmul(
                                scd[:, TOKH * j:TOKH * (j + 1)],
                                kF[b][hb:hb + DH, r2, P * j:P * (j + 1)],
                                qF[b][hb:hb + DH, c1, :],
                                start=False, stop=True,
                                skip_group_check=True)
                        wd = wp.tile([P, 2 * TOK], f8, tag="w")
                        with nc.allow_low_precision(reason="probs fp8"):
                            nc.scalar.activation(wd[:, 0:TOK], scd[:, 0:TOK],
                                                 AF.Exp, scale=0.125)
                        nc.tensor.matmul(
                            acc[:, TOKH:TOK], vF[b][:, r2, h],
                            wd[:, 0:TOK].rearrange("p (t c) -> p t c", t=2),
                            start=False, stop=True,
                            perf_mode=DR, skip_group_check=True)
                        # epilogue: normalize pair and store fp8
                        rcp = sqp.tile([DH, TOK], f32, tag="rcp")
                        with nc.allow_low_precision(reason="softmax recip"):
                            nc.vector.reciprocal(rcp, acc[DH:2 * DH, :])
                        with nc.allow_low_precision(reason="attn out fp8"):
                            nc.vector.tensor_tensor(
                                out=aT[b][hb:hb + DH, c0:c0 + 2, :].rearrange(
                                    "p c q -> p (c q)"),
                                in0=acc[0:DH, :], in1=rcp, op=OP.mult)

        def send_aT(b):
            with nc.named_scope(f"cc2_{b}"):
                nc.sync.dma_start(
                    cc2_in[b][:].rearrange("r p c -> p r c"), aT[b])
                nc.gpsimd.collective_compute(
                    "AllToAll", OP.bypass, replica_groups=RG,
                    ins=[cc2_in[b][:].opt()], outs=[cc2_out[b][:].opt()])

        def recv_aF(b):
            aF[b] = ares.tile([P, NCORES, TOKH], f8, name=f"aF{b}")
            nc.gpsimd.dma_start(
                aF[b], cc2_out[b][:].rearrange("r p c -> p r c"))

        h1T = res.tile([P, KD, TOK], f32r)

        def emit_proj(b, f2_list=None):
            tb = slice(TOKH * b, TOKH * (b + 1))
            with nc.named_scope(f"proj{b}"):
                for f2 in (range(KD // 2) if f2_list is None else f2_list):
                    pp = psum.tile([P, TOK], f32, tag="qk", bufs=2)
                    for half in range(2):
                        f = 2 * f2 + half
                        for rr in range(NCORES // 2):
                            nc.tensor.matmul(
                                pp[:, TOKH * half:TOKH * (half + 1)],
                                pw_sb[:, 2 * rr:2 * rr + 2, P * f:P * (f + 1)],
                                aF[b][:, 2 * rr:2 * rr + 2, :],
                                start=(rr == 0), stop=(rr == NCORES // 2 - 1),
                                perf_mode=DR, skip_group_check=True)
                    for half in range(2):
                        f = 2 * f2 + half
                        t1 = lnt.tile([P, TOKH], f32, tag="pj")
                        nc.vector.tensor_scalar(
                            out=t1, in0=pp[:, TOKH * half:TOKH * (half + 1)],
                            scalar1=1.0 / 256.0, scalar2=pb_sb[:, f:f + 1],
                            op0=OP.mult, op1=OP.add)
                        nc.vector.tensor_tensor(out=h1T[:, f, tb], in0=t1,
                                                in1=xT_sb[:, f, tb], op=OP.add)

        # ---- phase 4/5: pipelined attention + collectives + proj ----
        recv_qkv(0)
        recv_qkv(1)
        emit_attn(0)
        send_aT(0)
        recv_aF(0)

        def proj0_chunk(qcp):
            # slip proj(b0) f2-chunks into attn(b1)'s stream once aF[0]
            # is safely available (cc2_0 lands early in attn(b1))
            if qcp >= 2:
                emit_proj(0, f2_list=[2 * (qcp - 2), 2 * (qcp - 2) + 1])

        emit_attn(1, post_qc=proj0_chunk)
        send_aT(1)
        recv_aF(1)

        # ---- phase 6: per-batch LN2 -> mT (bf16); b0 overlaps cc2_1 ----
        mT = res.tile([P, KD, TOK], bf16)
        with nc.named_scope("ln2a"):
            emit_ln(lambda k: h1T[:, k, :], lambda k: mT[:, k, :], "ln2",
                    slice(0, TOKH), TOKH)
        emit_proj(1)
        with nc.named_scope("ln2b"):
            emit_ln(lambda k: h1T[:, k, :], lambda k: mT[:, k, :], "ln2",
                    slice(TOKH, TOK), TOKH)

        s1.close()  # release attention-era SBUF
        mlp = ctx.enter_context(tc.tile_pool(name="mlp", bufs=1))
        wgt = ctx.enter_context(tc.tile_pool(name="wgt", bufs=1))
        outp = ctx.enter_context(tc.tile_pool(name="outp", bufs=2))

        # ---- phase 7: MLP ----
        hT = mlp.tile([P, KDI, TOK], bf16)
        with nc.named_scope("fc1"):
            for j in range(KDI):
                fwt = wgt.tile([P, KD, P], bf16, tag="fw", bufs=3)
                nc.sync.dma_start(fwt, fw[j])
                ps = psum.tile([P, TOK], f32, tag="sc", bufs=2)
                for k in range(KD):
                    nc.tensor.matmul(ps, fwt[:, k, :], mT[:, k, :],
                                     start=(k == 0), stop=(k == KD - 1))
                nc.scalar.activation(hT[:, j, :], ps, AF.Gelu_apprx_tanh,
                                     bias=fb_sb[:, j:j + 1])
        with nc.named_scope("fc2"):
            for f in range(KD):
                gwt = wgt.tile([P, KDI, P], bf16, tag="gw", bufs=2)
                nc.sync.dma_start(gwt, gw[f])
                ps = psum.tile([P, TOK], f32, tag="sc", bufs=2)
                for k in range(KDI):
                    nc.tensor.matmul(ps, gwt[:, k, :], hT[:, k, :],
                                     start=(k == 0), stop=(k == KDI - 1))
                o = outp.tile([P, TOK], f32, tag="ot")
                nc.vector.tensor_scalar_add(o, ps, gb_sb[:, f:f + 1])
                nc.vector.tensor_tensor(out=o, in0=o, in1=h1T[:, f, :],
                                        op=OP.add)
                nc.sync.dma_start(outT[P * f:P * (f + 1), :], o)

    nc.compile()
    return nc


def shard_inputs(inputs):
    """Full inputs -> list of 8 per-core input dicts (host-side layout only)."""
    bf16 = ml_dtypes.bfloat16
    f8 = ml_dtypes.float8_e4m3
    f32 = np.float32
    hs = np.asarray(inputs["hidden_states"], f32)           # [B, S, D]
    l1g = np.asarray(inputs["ln1_g"], f32)
    l1b = np.asarray(inputs["ln1_b"], f32)
    l2g = np.asarray(inputs["ln2_g"], f32)
    l2b = np.asarray(inputs["ln2_b"], f32)
    attn_w = np.asarray(inputs["attn_w"], f32)
    attn_b = np.asarray(inputs["attn_b"], f32)
    fc_w = np.asarray(inputs["fc_w"], f32)
    fc_b = np.asarray(inputs["fc_b"], f32)

    # fold LN gains/biases into the downstream weights
    aw_f = l1g[:, None] * attn_w
    ab_f = attn_b + l1b @ attn_w
    fw_f = l2g[:, None] * fc_w
    fb_f = fc_b + l2b @ fc_w

    aw8 = np.ascontiguousarray(
        (aw_f * 256.0).reshape(KD, P, 3 * D).transpose(1, 0, 2).astype(f8))
    qkb = np.ascontiguousarray(
        ab_f[:2 * D].reshape(2, NCORES, P).transpose(2, 0, 1))
    pw8 = np.ascontiguousarray(
        (np.asarray(inputs["proj_w"], f32) * 256.0).reshape(KD, P, D)
        .transpose(1, 0, 2).astype(f8))
    bvf = ab_f[2 * D:3 * D]
    pb_f = np.asarray(inputs["proj_b"], f32) + bvf @ np.asarray(
        inputs["proj_w"], f32)
    pb = np.ascontiguousarray(pb_f.reshape(KD, P).T)
    fw = np.ascontiguousarray(
        fw_f.reshape(KD, P, KDI, P).transpose(2, 1, 0, 3).astype(bf16))
    fbv = np.ascontiguousarray(fb_f.reshape(KDI, P).T)
    gw = np.ascontiguousarray(
        np.asarray(inputs["fc2_w"], f32).reshape(KDI, P, KD, P)
        .transpose(2, 1, 0, 3).astype(bf16))
    gbv = np.ascontiguousarray(
        np.asarray(inputs["fc2_b"], f32).reshape(KD, P).T)

    ii, jj = np.meshgrid(np.arange(P), np.arange(TOKH), indexing="ij")
    mkm = np.stack([(jj < ii), (jj < ii + P)]).astype(f32) * -200.0
    mkm = np.ascontiguousarray(mkm.transpose(1, 0, 2)).astype(f8)
    mkdm = np.concatenate([mkm.astype(f32),
                           np.zeros((P, 2, TOKH), f32)], axis=2).astype(f8)
    idm = np.ascontiguousarray(np.eye(P, dtype=f8))

    maps = []
    for c in range(NCORES):
        xc = np.concatenate([hs[b, TOKH * c:TOKH * (c + 1)] for b in range(B)],
                            axis=0)                          # [TOK, D]
        xT_c = np.ascontiguousarray(xc.T)
        maps.append({
            "xT": xT_c, "aw": aw8, "qkb": qkb,
            "pw": pw8, "pb": pb, "fw": fw, "fb": fbv, "gw": gw, "gb": gbv,
            "mk": mkm, "mkd": mkdm, "idm": idm,
        })
    return maps


def unshard(results):
    out = np.empty((B, S, D), np.float32)
    for c, r in enumerate(results):
        o = np.asarray(r["outT"]).T                          # [TOK, D]
        for b in range(B):
            out[b, TOKH * c:TOKH * (c + 1)] = o[TOKH * b:TOKH * (b + 1)]
    return out


def kernel(**inputs):
    global _CACHED_NC
    from concourse.bass_utils import run_bass_kernel_spmd
    if _CACHED_NC is None:
        _CACHED_NC = build_nc()
    in_maps = shard_inputs(inputs)
    res = run_bass_kernel_spmd(_CACHED_NC, in_maps,
                               core_ids=list(range(NCORES)))
    return unshard(res.results)
